# revision 5
# baseline (speedup 1.0000x reference)
"""Trainium2 Bass kernel for ColumnConsistencyLoss (segment_reduce).

Problem: B=16, T=8192, C=128, N = B*T = 131072 tokens.
  probs p = softmax(logits, -1)            # (N, C)
  per column-id c: n_c = #valid, S_c = sum w*p, Q_c = sum w*p^2 (C x C)
  col_var_c = (sum_j Q_cj - sum_j S_cj^2 / n_c) / (n_c * C)
  loss = mean over columns with n_c > 1 of col_var_c

Sharding (v5): **by segment**.  Host sorts tokens by column id; core k
owns segments [16k, 16k+16) and receives exactly those tokens (padded
to 17408 = 136*128).  Per-core outputs cover disjoint segments, so the
cross-core reduction is a concat.  n_c comes from an exact host
bincount.  Rare overflow (> capacity tokens on one core) falls back to
exact host math for the excess tokens only.

Device kernel per core (tokens on partitions, token t = p*J + j):
  - DMA logits fp16 chunks [P, cj, C]; all scatter indices int16 once.
  - ScalarE: rhs[:, :, 0, :] = exp(L) -> bf16          (ACT is the
    bottleneck engine at ~1.23 ns/el; it does nothing else)
  - DVE+GpSimd: rhs[:, :, 1, :] = E^2  (TT mult split ~60/40 between
    the two engines; DVE side runs in 2x 16-bit mode)
  - DVE: d = rowsum(E) via 6-level pairwise-halving TT adds (2x mode),
    rm = 1/d via reciprocal_approx_fast
  - GpSimd: data[:, :, 0] = rm -> bf16, data[:, :, 1] = rm^2;
    local_scatter builds Mp[P, cj*32] with rho=1/d at slot
    (j*32 + lseg) and rho^2 at (j*32 + 16 + lseg); w/padding ride as
    idx=-1 (skipped -> zero row).
  - PE: psum_k[32, 256] += Mp[:, jj*32:+32]^T @ rhs[:, jj, :, :]
      rows 0:16  x cols 0:128  = S   (sum w/d * E   = sum w p)
      rows 16:32 x cols 128:256 = Q  (sum w/d^2 * E^2 = sum w p^2)
    (the other two quadrants are unused by the host)
Host: sums chunk psums, concats cores, finishes in fp64.
"""

import numpy as np

NCORES = 8
P = 128            # partitions
C = 128            # columns / segments
S16 = C // NCORES                  # 16 segments per core
B, T = 16, 8192
N_TOK = B * T
J = 136                            # token-cols per core (padded)
TOKCAP = J * P                     # 17408 tokens per core
CHUNKS = (8, 24, 32, 32, 24, 16)   # token-cols per chunk (sum = J)
W32 = 2 * S16                      # lhsT width: rho | rho^2 one-hots

TRACE = False
TRACE_TMPDIR = None
LAST_RESULT = None

_NC_CACHE = {}


def build_nc(chunks=CHUNKS):
    """Build + compile the Bass program (SPMD; same NEFF on all cores)."""
    from concourse import bacc, mybir
    import concourse.tile as tile

    f32 = mybir.dt.float32
    f16 = mybir.dt.float16
    bf16 = mybir.dt.bfloat16
    i16 = mybir.dt.int16
    Exp = mybir.ActivationFunctionType.Exp
    Alu = mybir.AluOpType

    j_full = sum(chunks)
    assert j_full == J
    nchunk = len(chunks)

    nc = bacc.Bacc("TRN2", target_bir_lowering=False, debug=False,
                   enable_asserts=False)

    lg_d = nc.dram_tensor("lg", [TOKCAP, C], f16, kind="ExternalInput")
    ix_d = nc.dram_tensor("ix", [2 * TOKCAP], i16, kind="ExternalInput")
    out_d = nc.dram_tensor("out", [W32, nchunk, 2 * C], f32,
                           kind="ExternalOutput")

    with tile.TileContext(nc) as tc:
        with (
            tc.tile_pool(name="const", bufs=1) as constp,
            tc.tile_pool(name="ld", bufs=4) as ldp,
            tc.tile_pool(name="big", bufs=4) as bigp,
            tc.tile_pool(name="small", bufs=4) as smallp,
            tc.tile_pool(name="psum", bufs=1, space="PSUM") as psump,
        ):
            psums = [psump.tile([W32, 2 * C], f32, name=f"ps{k}")
                     for k in range(nchunk)]

            lg_ap = lg_d[:].rearrange("(p j) c -> p j c", j=j_full)
            ix_ap = ix_d[:].rearrange("(p q) -> p q", q=2 * j_full)

            # all scatter indices in one small upfront DMA
            ixt = constp.tile([P, 2 * j_full], i16)
            nc.sync.dma_start(ixt[:], ix_ap)
            out_t = constp.tile([W32, nchunk, 2 * C], f32)

            offs = [sum(chunks[:k]) for k in range(nchunk)]
            Ls = [None] * nchunk
            RHs = [None] * nchunk

            def emit_load(k):
                cj = chunks[k]
                L = ldp.tile([P, cj, C], f16, tag="L")
                nc.sync.dma_start(L[:], lg_ap[:, offs[k]:offs[k] + cj, :])
                Ls[k] = L

            def halves(cj):
                if cj >= 16:
                    return [(0, cj // 2), (cj // 2, cj)]
                return [(0, cj)]

            def emit_exp(k):
                cj = chunks[k]
                rhs = bigp.tile([P, cj, 2, C], bf16, tag="rhs")
                for a, b in halves(cj):
                    nc.scalar.activation(rhs[:, a:b, 0, :], Ls[k][:, a:b, :],
                                         Exp)
                RHs[k] = rhs

            emit_load(0)
            emit_load(1)
            emit_exp(0)
            for k, cj in enumerate(chunks):
                if k + 2 < nchunk:
                    emit_load(k + 2)
                rhs = RHs[k]
                E = rhs[:, :, 0, :]

                # E^2 into rhs[:, :, 1, :]: split DVE (2x mode) / GpSimd
                s = max(2, int(round(cj * 0.62)) & ~1)
                nc.vector.tensor_tensor(rhs[:, 0:s, 1, :], rhs[:, 0:s, 0, :],
                                        rhs[:, 0:s, 0, :], op=Alu.mult)
                nc.gpsimd.tensor_tensor(rhs[:, s:cj, 1, :],
                                        rhs[:, s:cj, 0, :],
                                        rhs[:, s:cj, 0, :], op=Alu.mult)

                # d = rowsum(E) by pairwise halving (bf16 2x), then 1/d
                h1 = smallp.tile([P, cj, 64], bf16, tag="h1")
                nc.vector.tensor_tensor(h1[:], E[:, :, 0:64], E[:, :, 64:128],
                                        op=Alu.add)
                if k + 1 < nchunk:
                    emit_exp(k + 1)
                h2 = smallp.tile([P, cj, 32], bf16, tag="h2")
                nc.vector.tensor_tensor(h2[:], h1[:, :, 0:32], h1[:, :, 32:64],
                                        op=Alu.add)
                h3 = smallp.tile([P, cj, 16], bf16, tag="h3")
                nc.vector.tensor_tensor(h3[:], h2[:, :, 0:16], h2[:, :, 16:32],
                                        op=Alu.add)
                h4 = smallp.tile([P, cj, 8], bf16, tag="h4")
                nc.vector.tensor_tensor(h4[:], h3[:, :, 0:8], h3[:, :, 8:16],
                                        op=Alu.add)
                h5 = smallp.tile([P, cj, 4], bf16, tag="h5")
                nc.vector.tensor_tensor(h5[:], h4[:, :, 0:4], h4[:, :, 4:8],
                                        op=Alu.add)
                h6 = smallp.tile([P, cj, 2], bf16, tag="h6")
                nc.vector.tensor_tensor(h6[:], h5[:, :, 0:2], h5[:, :, 2:4],
                                        op=Alu.add)
                d32 = smallp.tile([P, cj], f32, tag="d32")
                nc.vector.tensor_tensor(d32[:], h6[:, :, 0], h6[:, :, 1],
                                        op=Alu.add)
                rm = smallp.tile([P, cj], f32, tag="rm")
                nc.vector.reciprocal_approx_fast(rm[:], d32[:])

                # scatter data: rho = 1/d (bf16), rho^2
                data = smallp.tile([P, cj, 2], bf16, tag="data")
                nc.gpsimd.tensor_copy(data[:, :, 0], rm[:])
                nc.gpsimd.tensor_tensor(data[:, :, 1], data[:, :, 0],
                                        data[:, :, 0], op=Alu.mult)
                Mp = smallp.tile([P, cj * W32], bf16, tag="Mp")
                nc.gpsimd.local_scatter(
                    Mp[:], data[:].rearrange("p a b -> p (a b)"),
                    ixt[:, 2 * offs[k]:2 * (offs[k] + cj)],
                    channels=P, num_elems=cj * W32, num_idxs=2 * cj)

                for jj in range(cj):
                    nc.tensor.matmul(
                        psums[k][:], Mp[:, jj * W32:(jj + 1) * W32],
                        rhs[:, jj, :, :], start=(jj == 0), stop=(jj == cj - 1))
                nc.vector.tensor_copy(out_t[:, k, :], psums[k][:])

            nc.sync.dma_start(out_d[:], out_t[:])

    nc.compile()
    return nc


def _get_nc():
    key = CHUNKS
    if key not in _NC_CACHE:
        _NC_CACHE[key] = build_nc(key)
    return _NC_CACHE[key]


def kernel(column_logits, column_assignments, valid_mask):
    global LAST_RESULT
    from concourse.bass_utils import run_bass_kernel_spmd

    logits = np.asarray(column_logits, dtype=np.float32).reshape(N_TOK, C)
    seg = np.asarray(column_assignments).reshape(N_TOK).astype(np.int64)
    w = np.asarray(valid_mask).reshape(N_TOK).astype(bool)

    order = np.argsort(seg, kind="stable")
    seg_sorted = seg[order]
    bounds = np.searchsorted(seg_sorted, np.arange(0, C + 1, S16))

    # scatter-slot template: position i (= p*J + j) -> slot of j's chunk
    offs = np.cumsum([0] + list(CHUNKS))
    jj = np.arange(TOKCAP) % J
    chunk_of = np.searchsorted(offs, jj, side="right") - 1
    base = ((jj - offs[chunk_of]) * W32).astype(np.int16)

    in_maps = []
    S_extra = np.zeros((C, C), np.float64)
    Q_extra = np.zeros((C, C), np.float64)
    for k in range(NCORES):
        tk = order[bounds[k]:bounds[k + 1]]
        if tk.size > TOKCAP:        # exact host fallback (never for seed 0)
            ov = tk[TOKCAP:]
            tk = tk[:TOKCAP]
            lo = logits[ov].astype(np.float64)
            pr = np.exp(lo - lo.max(axis=1, keepdims=True))
            pr /= pr.sum(axis=1, keepdims=True)
            for t, row in zip(ov, pr):
                if w[t]:
                    S_extra[seg[t]] += row
                    Q_extra[seg[t]] += row * row
        ck = tk.size
        lg16 = np.zeros((TOKCAP, C), dtype=np.float16)
        lg16[:ck] = logits[tk].astype(np.float16)
        ix = np.full((TOKCAP, 2), -1, dtype=np.int16)
        lseg = (seg[tk] - k * S16).astype(np.int16)
        ok = w[tk]
        ix[:ck, 0] = np.where(ok, base[:ck] + lseg, np.int16(-1))
        ix[:ck, 1] = np.where(ok, base[:ck] + np.int16(S16) + lseg,
                              np.int16(-1))
        in_maps.append({"lg": lg16, "ix": ix.reshape(-1)})

    nc = _get_nc()
    res = run_bass_kernel_spmd(nc, in_maps, list(range(NCORES)), trace=TRACE,
                               tmpdir=TRACE_TMPDIR)
    LAST_RESULT = res

    S = np.zeros((C, C), np.float64)
    Q = np.zeros((C, C), np.float64)
    for k, rmap in enumerate(res.results):
        o = np.asarray(rmap["out"], dtype=np.float64).sum(axis=1)  # [32, 256]
        sl = slice(k * S16, (k + 1) * S16)
        S[sl] = o[0:S16, 0:C]
        Q[sl] = o[S16:W32, C:2 * C]
    S += S_extra
    Q += Q_extra

    n = np.bincount(seg[w], minlength=C).astype(np.float64)
    n_safe = np.maximum(n, 1.0)
    ssd_sum = Q.sum(axis=1) - (S * S).sum(axis=1) / n_safe
    col_var = ssd_sum / (n_safe * C)
    has_multi = n > 1.0
    count = has_multi.sum()
    total = np.where(has_multi, col_var, 0.0).sum()
    loss = total / max(count, 1.0) if count > 0 else 0.0
    return np.asarray(loss, dtype=np.float32)


# revision 7
# speedup vs baseline: 1.1405x; 1.1405x over previous
"""Trainium2 Bass kernel for ColumnConsistencyLoss (segment_reduce).

Problem: B=16, T=8192, C=128, N = B*T = 131072 tokens.
  probs p = softmax(logits, -1)            # (N, C)
  per column-id c: n_c = #valid, S_c = sum w*p, Q_c = sum w*p^2 (C x C)
  col_var_c = (sum_j Q_cj - sum_j S_cj^2 / n_c) / (n_c * C)
  loss = mean over columns with n_c > 1 of col_var_c

Sharding (v5): **by segment**.  Host sorts tokens by column id; core k
owns segments [16k, 16k+16) and receives exactly those tokens (padded
to 17408 = 136*128).  Per-core outputs cover disjoint segments, so the
cross-core reduction is a concat.  n_c comes from an exact host
bincount.  Rare overflow (> capacity tokens on one core) falls back to
exact host math for the excess tokens only.

Device kernel per core (tokens on partitions, token t = p*J + j):
  - DMA logits fp16 chunks [P, cj, C]; all scatter indices int16 once.
  - ScalarE: rhs[:, :, 0, :] = exp(L) -> bf16          (ACT is the
    bottleneck engine at ~1.23 ns/el; it does nothing else)
  - DVE+GpSimd: rhs[:, :, 1, :] = E^2  (TT mult split ~60/40 between
    the two engines; DVE side runs in 2x 16-bit mode)
  - DVE: d = rowsum(E) via 6-level pairwise-halving TT adds (2x mode),
    rm = 1/d via reciprocal_approx_fast
  - GpSimd: data[:, :, 0] = rm -> bf16, data[:, :, 1] = rm^2;
    local_scatter builds Mp[P, cj*32] with rho=1/d at slot
    (j*32 + lseg) and rho^2 at (j*32 + 16 + lseg); w/padding ride as
    idx=-1 (skipped -> zero row).
  - PE: psum_k[32, 256] += Mp[:, jj*32:+32]^T @ rhs[:, jj, :, :]
      rows 0:16  x cols 0:128  = S   (sum w/d * E   = sum w p)
      rows 16:32 x cols 128:256 = Q  (sum w/d^2 * E^2 = sum w p^2)
    (the other two quadrants are unused by the host)
Host: sums chunk psums, concats cores, finishes in fp64.
"""

import numpy as np

NCORES = 8
P = 128            # partitions
C = 128            # columns / segments
S16 = C // NCORES                  # 16 segments per core
B, T = 16, 8192
N_TOK = B * T
J = 136                            # token-cols per core (padded)
TOKCAP = J * P                     # 17408 tokens per core
CHUNKS = (8, 24, 36, 36, 32)       # token-cols per chunk (sum = J)
W32 = 2 * S16                      # lhsT width: rho | rho^2 one-hots

TRACE = False
TRACE_TMPDIR = None
LAST_RESULT = None

_NC_CACHE = {}


def build_nc(chunks=CHUNKS):
    """Build + compile the Bass program (SPMD; same NEFF on all cores)."""
    from concourse import bacc, mybir
    import concourse.tile as tile

    f32 = mybir.dt.float32
    f16 = mybir.dt.float16
    bf16 = mybir.dt.bfloat16
    i16 = mybir.dt.int16
    Exp = mybir.ActivationFunctionType.Exp
    Alu = mybir.AluOpType

    j_full = sum(chunks)
    assert j_full == J
    nchunk = len(chunks)

    nc = bacc.Bacc("TRN2", target_bir_lowering=False, debug=False,
                   enable_asserts=False)

    lg_d = nc.dram_tensor("lg", [TOKCAP, C], f16, kind="ExternalInput")
    ix_d = nc.dram_tensor("ix", [2 * TOKCAP], i16, kind="ExternalInput")
    out_d = nc.dram_tensor("out", [W32, nchunk, 2 * C], f32,
                           kind="ExternalOutput")

    with tile.TileContext(nc) as tc:
        with (
            tc.tile_pool(name="const", bufs=1) as constp,
            tc.tile_pool(name="ld", bufs=4) as ldp,
            tc.tile_pool(name="big", bufs=4) as bigp,
            tc.tile_pool(name="small", bufs=4) as smallp,
            tc.tile_pool(name="psum", bufs=1, space="PSUM") as psump,
        ):
            psums = [psump.tile([W32, 2 * C], f32, name=f"ps{k}")
                     for k in range(nchunk)]

            lg_ap = lg_d[:].rearrange("(p j) c -> p j c", j=j_full)
            ix_ap = ix_d[:].rearrange("(p q) -> p q", q=2 * j_full)

            # all scatter indices in one small upfront DMA
            ixt = constp.tile([P, 2 * j_full], i16)
            nc.sync.dma_start(ixt[:], ix_ap)
            out_t = constp.tile([W32, nchunk, 2 * C], f32)

            offs = [sum(chunks[:k]) for k in range(nchunk)]
            Ls = [None] * nchunk
            RHs = [None] * nchunk

            def emit_load(k):
                cj = chunks[k]
                L = ldp.tile([P, cj, C], f16, tag="L")
                nc.sync.dma_start(L[:], lg_ap[:, offs[k]:offs[k] + cj, :])
                Ls[k] = L

            def halves(cj):
                if cj >= 16:
                    return [(0, cj // 2), (cj // 2, cj)]
                return [(0, cj)]

            def emit_exp(k):
                cj = chunks[k]
                rhs = bigp.tile([P, cj, 2, C], bf16, tag="rhs")
                for a, b in halves(cj):
                    nc.scalar.activation(rhs[:, a:b, 0, :], Ls[k][:, a:b, :],
                                         Exp)
                RHs[k] = rhs

            emit_load(0)
            emit_load(1)
            emit_exp(0)
            for k, cj in enumerate(chunks):
                if k + 2 < nchunk:
                    emit_load(k + 2)
                rhs = RHs[k]
                E = rhs[:, :, 0, :]

                # E^2 into rhs[:, :, 1, :] (DVE, bf16 2x mode)
                nc.vector.tensor_tensor(rhs[:, :, 1, :], E, E, op=Alu.mult)

                # d = rowsum(E) by pairwise halving (bf16 2x); the narrow
                # tail levels go to the otherwise-idle GpSimd engine
                h1 = smallp.tile([P, cj, 64], bf16, tag="h1")
                nc.vector.tensor_tensor(h1[:], E[:, :, 0:64], E[:, :, 64:128],
                                        op=Alu.add)
                if k + 1 < nchunk:
                    emit_exp(k + 1)
                h2 = smallp.tile([P, cj, 32], bf16, tag="h2")
                nc.vector.tensor_tensor(h2[:], h1[:, :, 0:32], h1[:, :, 32:64],
                                        op=Alu.add)
                h3 = smallp.tile([P, cj, 16], bf16, tag="h3")
                nc.vector.tensor_tensor(h3[:], h2[:, :, 0:16], h2[:, :, 16:32],
                                        op=Alu.add)
                h4 = smallp.tile([P, cj, 8], bf16, tag="h4")
                nc.vector.tensor_tensor(h4[:], h3[:, :, 0:8], h3[:, :, 8:16],
                                        op=Alu.add)
                h5 = smallp.tile([P, cj, 4], bf16, tag="h5")
                nc.gpsimd.tensor_tensor(h5[:], h4[:, :, 0:4], h4[:, :, 4:8],
                                        op=Alu.add)
                h6 = smallp.tile([P, cj, 2], bf16, tag="h6")
                nc.gpsimd.tensor_tensor(h6[:], h5[:, :, 0:2], h5[:, :, 2:4],
                                        op=Alu.add)
                d32 = smallp.tile([P, cj], f32, tag="d32")
                nc.gpsimd.tensor_tensor(d32[:], h6[:, :, 0], h6[:, :, 1],
                                        op=Alu.add)
                rm = smallp.tile([P, cj], f32, tag="rm")
                nc.vector.reciprocal_approx_fast(rm[:], d32[:])

                # scatter data: rho = 1/d (bf16), rho^2
                data = smallp.tile([P, cj, 2], bf16, tag="data")
                nc.gpsimd.tensor_copy(data[:, :, 0], rm[:])
                nc.gpsimd.tensor_tensor(data[:, :, 1], data[:, :, 0],
                                        data[:, :, 0], op=Alu.mult)
                Mp = smallp.tile([P, cj * W32], bf16, tag="Mp")
                nc.gpsimd.local_scatter(
                    Mp[:], data[:].rearrange("p a b -> p (a b)"),
                    ixt[:, 2 * offs[k]:2 * (offs[k] + cj)],
                    channels=P, num_elems=cj * W32, num_idxs=2 * cj)

                for jj in range(cj):
                    nc.tensor.matmul(
                        psums[k][:], Mp[:, jj * W32:(jj + 1) * W32],
                        rhs[:, jj, :, :], start=(jj == 0), stop=(jj == cj - 1))
                nc.vector.tensor_copy(out_t[:, k, :], psums[k][:])

            nc.sync.dma_start(out_d[:], out_t[:])

    nc.compile()
    return nc


def _get_nc():
    key = CHUNKS
    if key not in _NC_CACHE:
        _NC_CACHE[key] = build_nc(key)
    return _NC_CACHE[key]


def kernel(column_logits, column_assignments, valid_mask):
    global LAST_RESULT
    from concourse.bass_utils import run_bass_kernel_spmd

    logits = np.asarray(column_logits, dtype=np.float32).reshape(N_TOK, C)
    seg = np.asarray(column_assignments).reshape(N_TOK).astype(np.int64)
    w = np.asarray(valid_mask).reshape(N_TOK).astype(bool)

    order = np.argsort(seg, kind="stable")
    seg_sorted = seg[order]
    bounds = np.searchsorted(seg_sorted, np.arange(0, C + 1, S16))

    # scatter-slot template: position i (= p*J + j) -> slot of j's chunk
    offs = np.cumsum([0] + list(CHUNKS))
    jj = np.arange(TOKCAP) % J
    chunk_of = np.searchsorted(offs, jj, side="right") - 1
    base = ((jj - offs[chunk_of]) * W32).astype(np.int16)

    in_maps = []
    S_extra = np.zeros((C, C), np.float64)
    Q_extra = np.zeros((C, C), np.float64)
    for k in range(NCORES):
        tk = order[bounds[k]:bounds[k + 1]]
        if tk.size > TOKCAP:        # exact host fallback (never for seed 0)
            ov = tk[TOKCAP:]
            tk = tk[:TOKCAP]
            lo = logits[ov].astype(np.float64)
            pr = np.exp(lo - lo.max(axis=1, keepdims=True))
            pr /= pr.sum(axis=1, keepdims=True)
            for t, row in zip(ov, pr):
                if w[t]:
                    S_extra[seg[t]] += row
                    Q_extra[seg[t]] += row * row
        ck = tk.size
        lg16 = np.zeros((TOKCAP, C), dtype=np.float16)
        lg16[:ck] = logits[tk].astype(np.float16)
        ix = np.full((TOKCAP, 2), -1, dtype=np.int16)
        lseg = (seg[tk] - k * S16).astype(np.int16)
        ok = w[tk]
        ix[:ck, 0] = np.where(ok, base[:ck] + lseg, np.int16(-1))
        ix[:ck, 1] = np.where(ok, base[:ck] + np.int16(S16) + lseg,
                              np.int16(-1))
        in_maps.append({"lg": lg16, "ix": ix.reshape(-1)})

    nc = _get_nc()
    res = run_bass_kernel_spmd(nc, in_maps, list(range(NCORES)), trace=TRACE,
                               tmpdir=TRACE_TMPDIR)
    LAST_RESULT = res

    S = np.zeros((C, C), np.float64)
    Q = np.zeros((C, C), np.float64)
    for k, rmap in enumerate(res.results):
        o = np.asarray(rmap["out"], dtype=np.float64).sum(axis=1)  # [32, 256]
        sl = slice(k * S16, (k + 1) * S16)
        S[sl] = o[0:S16, 0:C]
        Q[sl] = o[S16:W32, C:2 * C]
    S += S_extra
    Q += Q_extra

    n = np.bincount(seg[w], minlength=C).astype(np.float64)
    n_safe = np.maximum(n, 1.0)
    ssd_sum = Q.sum(axis=1) - (S * S).sum(axis=1) / n_safe
    col_var = ssd_sum / (n_safe * C)
    has_multi = n > 1.0
    count = has_multi.sum()
    total = np.where(has_multi, col_var, 0.0).sum()
    loss = total / max(count, 1.0) if count > 0 else 0.0
    return np.asarray(loss, dtype=np.float32)
